# revision 5
# baseline (speedup 1.0000x reference)
"""Causal attention (B=8, S=2048, D=1024, d_k=d_v=512) on 8 TRN2 NeuronCores.

Sharding: data-parallel over batch — each core computes one batch element's
full attention. Weights are replicated. No collectives.

Per-core pipeline (all matmuls in float32r — full PE rate, ~1.5e-4 rel err):
  A) W^T tiles via PE transpose        [d, k] layout for projections
  B) Xkv^T via PE transpose -> K^T, V  (two halves to bound SBUF)
  C) Xq^T via PE transpose -> Q^T (scaled by 1/sqrt(d_k) at copyback)
  D) per q-tile i: S = Q^T.T @ K^T chunks into PSUM (causal: only s <= (i+1)*128),
     diagonal causal mask added via identity@mask matmul into PSUM,
     copy to SBUF (ACT), row-max (DVE, negated), exp+rowsum fused (ACT),
     P^T via PE transpose, O = P^T.T @ V accumulated in PSUM,
     O row-normalized by 1/rowsum at copyback, DMA out.
"""

import numpy as np

import concourse.bacc as bacc
import concourse.tile as tile
from concourse import mybir
from concourse.bass_utils import run_bass_kernel_spmd
from concourse.masks import make_identity

P = 128
S, D, DK, DV = 2048, 1024, 512, 512
ST, DT, KT = S // P, D // P, DK // P
SCALE = float(DK) ** -0.5
NEG = -30000.0
N_CORES = 8

F32 = mybir.dt.float32
F32R = mybir.dt.float32r


def _build():
    nc = bacc.Bacc(None, target_bir_lowering=False)
    xq_d = nc.declare_dram_parameter("xq", [S, D], F32, isOutput=False)
    xkv_d = nc.declare_dram_parameter("xkv", [S, D], F32, isOutput=False)
    w_d = {
        name: nc.declare_dram_parameter(name, [DK, D], F32, isOutput=False)
        for name in ("wq", "wk", "wv")
    }
    out_d = nc.declare_dram_parameter("out", [S, DV], F32, isOutput=True)

    with tile.TileContext(nc) as tc:
        with (
            tc.tile_pool(name="consts", bufs=1) as consts,
            tc.tile_pool(name="psum", bufs=1, space="PSUM") as psum,
            tc.tile_pool(name="kv", bufs=1) as kv_pool,
            tc.tile_pool(name="q", bufs=1) as q_pool,
        ):
            ident32 = consts.tile([P, P], F32, tag="ident32")
            make_identity(nc, ident32)
            ident_r = consts.tile([P, P], F32R, tag="ident_r")
            nc.vector.tensor_copy(ident_r, ident32)
            # causal mask for the diagonal block: 0 on/below diag, NEG above
            mask32 = consts.tile([P, P], F32, tag="mask32")
            nc.gpsimd.memset(mask32, 0.0)
            nc.gpsimd.affine_select(
                out=mask32, in_=mask32, compare_op=mybir.AluOpType.is_ge,
                fill=NEG, base=0, pattern=[[-1, P]], channel_multiplier=1,
            )
            mask_r = consts.tile([P, P], F32R, tag="mask_r")
            nc.vector.tensor_copy(mask_r, mask32)

            kT = kv_pool.tile([P, KT, S], F32R, tag="kT")      # K^T: [k_part, kt, s]
            v_sb = kv_pool.tile([P, ST, DV], F32R, tag="v")    # V: [s_part, st, v]
            qT = q_pool.tile([P, KT, S], F32R, tag="qT")       # Q^T: [k_part, kt, q]

            def ps_tile(tag, w, dt):
                return psum.tile([P, w], dt, tag=tag, name=tag)

            # ---- Phase A: weight transposes -> wT[d_part, dt, k] ----
            with (
                tc.tile_pool(name="wkv", bufs=1) as wkv_pool,
                tc.tile_pool(name="wq", bufs=1) as wq_pool,
                tc.tile_pool(name="wstage", bufs=2) as wstage,
            ):
                wT = {
                    "wq": wq_pool.tile([P, DT, DK], F32R, tag="wqT", name="wqT"),
                    "wk": wkv_pool.tile([P, DT, DK], F32R, tag="wkT", name="wkT"),
                    "wv": wkv_pool.tile([P, DT, DK], F32R, tag="wvT", name="wvT"),
                }
                for name in ("wk", "wv", "wq"):
                    for kt in range(KT):
                        wn = wstage.tile([P, D], F32, tag="wnat")
                        nc.sync.dma_start(out=wn, in_=w_d[name][kt * P:(kt + 1) * P, :])
                        for dt_ in range(DT):
                            ps = ps_tile("tp", P, F32)
                            nc.tensor.transpose(ps, wn[:, dt_ * P:(dt_ + 1) * P], ident32)
                            nc.vector.tensor_copy(
                                wT[name][:, dt_, kt * P:(kt + 1) * P], ps
                            )

                # ---- Phases B/C: X^T + projections ----
                with tc.tile_pool(name="xstage", bufs=3) as xstage:

                    def transpose_x(x_dram, h, xT):
                        """Transpose rows [h*1024, (h+1)*1024) of x into xT[d_part, dt, 1024]."""
                        for sl in range(ST // 2):
                            st = h * (ST // 2) + sl
                            xn = xstage.tile([P, D], F32, tag="xnat")
                            nc.sync.dma_start(
                                out=xn, in_=x_dram[st * P:(st + 1) * P, :]
                            )
                            for dt_ in range(DT):
                                ps = ps_tile("tp", P, F32)
                                nc.tensor.transpose(
                                    ps, xn[:, dt_ * P:(dt_ + 1) * P], ident32
                                )
                                nc.vector.tensor_copy(
                                    xT[:, dt_, sl * P:(sl + 1) * P], ps
                                )

                    # Phase B: K^T and V from Xkv, in two halves
                    for h in range(2):
                        xT = xstage.tile([P, DT, S // 2], F32R, tag="xT", bufs=1)
                        transpose_x(xkv_d, h, xT)
                        # K^T[kt][:, h*1024 + c*512 ...]
                        for kt in range(KT):
                            for c in range(2):
                                ps = ps_tile("mm", 512, F32)
                                for dt_ in range(DT):
                                    nc.tensor.matmul(
                                        ps,
                                        wT["wk"][:, dt_, kt * P:(kt + 1) * P],
                                        xT[:, dt_, c * 512:(c + 1) * 512],
                                        start=(dt_ == 0), stop=(dt_ == DT - 1),
                                    )
                                nc.scalar.copy(
                                    kT[:, kt, h * 1024 + c * 512: h * 1024 + (c + 1) * 512],
                                    ps,
                                )
                        # V rows
                        for sl in range(ST // 2):
                            st = h * (ST // 2) + sl
                            ps = ps_tile("mm", 512, F32)
                            for dt_ in range(DT):
                                nc.tensor.matmul(
                                    ps,
                                    xT[:, dt_, sl * P:(sl + 1) * P],
                                    wT["wv"][:, dt_, :],
                                    start=(dt_ == 0), stop=(dt_ == DT - 1),
                                )
                            nc.scalar.copy(v_sb[:, st, :], ps)

                    # Phase C: Q^T from Xq (scaled)
                    for h in range(2):
                        xT = xstage.tile([P, DT, S // 2], F32R, tag="xT", bufs=1)
                        transpose_x(xq_d, h, xT)
                        for kt in range(KT):
                            for c in range(2):
                                ps = ps_tile("mm", 512, F32)
                                for dt_ in range(DT):
                                    nc.tensor.matmul(
                                        ps,
                                        wT["wq"][:, dt_, kt * P:(kt + 1) * P],
                                        xT[:, dt_, c * 512:(c + 1) * 512],
                                        start=(dt_ == 0), stop=(dt_ == DT - 1),
                                    )
                                nc.scalar.mul(
                                    qT[:, kt, h * 1024 + c * 512: h * 1024 + (c + 1) * 512],
                                    ps, SCALE,
                                )

            # ---- Phase D: attention, per q-tile ----
            with tc.tile_pool(name="attn", bufs=2) as attn:
                state = {}

                def emit_scores(i):
                    L = (i + 1) * P
                    NL = (L + 511) // 512
                    s_t = attn.tile([P, S], F32, tag="s_sb")
                    for c in range(NL):
                        w = min(512, L - c * 512)
                        ps = ps_tile("mm", 512, F32)
                        last_chunk = c == NL - 1
                        for kt in range(KT):
                            nc.tensor.matmul(
                                ps[:, :w],
                                qT[:, kt, i * P:(i + 1) * P],
                                kT[:, kt, c * 512:c * 512 + w],
                                start=(kt == 0),
                                stop=(kt == KT - 1 and not last_chunk),
                            )
                        if last_chunk:
                            # add causal mask to the diagonal 128 cols via PE
                            nc.tensor.matmul(
                                ps[:, w - P:w], ident_r, mask_r,
                                start=False, stop=True,
                            )
                        nc.scalar.copy(s_t[:, c * 512:c * 512 + w], ps[:, :w])
                    nmx = attn.tile([P, 1], F32, tag="nmx")
                    nc.vector.tensor_reduce(
                        out=nmx, in_=s_t[:, :L], axis=mybir.AxisListType.X,
                        op=mybir.AluOpType.max, negate=True,
                    )
                    p_t = attn.tile([P, S], F32R, tag="p_sb")
                    rs = attn.tile([P, 1], F32, tag="rs")
                    nc.scalar.activation(
                        out=p_t[:, :L], in_=s_t[:, :L],
                        func=mybir.ActivationFunctionType.Exp,
                        bias=nmx, scale=1.0, accum_out=rs,
                    )
                    rinv = attn.tile([P, 1], F32, tag="rinv")
                    nc.vector.reciprocal(rinv, rs)
                    state[i] = (p_t, rinv)

                def emit_out(i):
                    p_t, rinv = state.pop(i)
                    pT = attn.tile([P, ST, P], F32R, tag="pT")
                    for st in range(i + 1):
                        ps = ps_tile("tp", P, F32R)
                        nc.tensor.transpose(
                            ps, p_t[:, st * P:(st + 1) * P], ident_r
                        )
                        nc.vector.tensor_copy(pT[:, st, :], ps)
                    ps_o = ps_tile("o", 512, F32)
                    for st in range(i + 1):
                        nc.tensor.matmul(
                            ps_o, pT[:, st, :], v_sb[:, st, :],
                            start=(st == 0), stop=(st == i),
                        )
                    o_t = attn.tile([P, DV], F32, tag="o_sb")
                    nc.scalar.activation(
                        out=o_t, in_=ps_o,
                        func=mybir.ActivationFunctionType.Copy, scale=rinv,
                    )
                    nc.sync.dma_start(out=out_d[i * P:(i + 1) * P, :], in_=o_t)

                for i in range(ST):
                    emit_scores(i)
                    if i > 0:
                        emit_out(i - 1)
                emit_out(ST - 1)

    nc.finalize()
    return nc


_NC = None


def _get_nc():
    global _NC
    if _NC is None:
        _NC = _build()
    return _NC


def kernel(source_query, source_key_value, source_query_padding_mask,
           source_key_value_padding_mask, Wq, Wk, Wv):
    nc = _get_nc()
    wq = np.ascontiguousarray(Wq, dtype=np.float32)
    wk = np.ascontiguousarray(Wk, dtype=np.float32)
    wv = np.ascontiguousarray(Wv, dtype=np.float32)
    in_maps = [
        {
            "xq": np.ascontiguousarray(source_query[c], dtype=np.float32),
            "xkv": np.ascontiguousarray(source_key_value[c], dtype=np.float32),
            "wq": wq, "wk": wk, "wv": wv,
        }
        for c in range(N_CORES)
    ]
    res = run_bass_kernel_spmd(nc, in_maps, list(range(N_CORES)))
    return np.stack([res.results[c]["out"] for c in range(N_CORES)]).astype(np.float32)


# revision 13
# speedup vs baseline: 3.3187x; 3.3187x over previous
"""Causal attention (B=8, S=2048, D=1024, d_k=d_v=512) on 8 TRN2 NeuronCores.

Sharding: data-parallel over batch — each core computes one batch element's
full attention. Weights are replicated. No collectives.

Per-core pipeline (all matmuls in float32r — full PE rate, ~1.5e-4 rel err):
  A) W^T tiles via PE transpose        [d, k] layout for projections
  B) Xkv^T via PE transpose -> K^T, V  (two halves to bound SBUF)
  C) Xq^T via PE transpose -> Q^T (scaled by 1/sqrt(d_k) at copyback)
  D) per q-tile i: S = Q^T.T @ K^T chunks into PSUM (causal: only s <= (i+1)*128),
     diagonal causal mask added via identity@mask matmul into PSUM,
     copy to SBUF (ACT), row-max (DVE, negated), exp+rowsum fused (ACT),
     P^T via PE transpose, O = P^T.T @ V accumulated in PSUM,
     O row-normalized by 1/rowsum at copyback, DMA out.
"""

import numpy as np

import concourse.bacc as bacc
import concourse.tile as tile
from concourse import mybir
from concourse.bass_utils import run_bass_kernel_spmd
from concourse.masks import make_identity

P = 128
S, D, DK, DV = 2048, 1024, 512, 512
ST, DT, KT = S // P, D // P, DK // P
SCALE = float(DK) ** -0.5
NEG = -30000.0
N_CORES = 8

F32 = mybir.dt.float32
F32R = mybir.dt.float32r


def _build():
    nc = bacc.Bacc(None, target_bir_lowering=False)
    xq_d = nc.declare_dram_parameter("xq", [S, D], F32, isOutput=False)
    xkv_d = nc.declare_dram_parameter("xkv", [S, D], F32, isOutput=False)
    w_d = {
        name: nc.declare_dram_parameter(name, [DK, D], F32, isOutput=False)
        for name in ("wq", "wk", "wv")
    }
    out_d = nc.declare_dram_parameter("out", [S, DV], F32, isOutput=True)

    with tile.TileContext(nc) as tc:
        with (
            tc.tile_pool(name="consts", bufs=1) as consts,
            tc.tile_pool(name="psum", bufs=1, space="PSUM") as psum,
            tc.tile_pool(name="kv", bufs=1) as kv_pool,
            tc.tile_pool(name="q", bufs=1) as q_pool,
        ):
            ident32 = consts.tile([P, P], F32, tag="ident32")
            make_identity(nc, ident32)
            ident_r = consts.tile([P, P], F32R, tag="ident_r")
            nc.vector.tensor_copy(ident_r, ident32)
            # causal mask for the diagonal block: 0 on/below diag, NEG above
            mask32 = consts.tile([P, P], F32, tag="mask32")
            nc.gpsimd.memset(mask32, 0.0)
            nc.gpsimd.affine_select(
                out=mask32, in_=mask32, compare_op=mybir.AluOpType.is_ge,
                fill=NEG, base=0, pattern=[[-1, P]], channel_multiplier=1,
            )
            mask_r = consts.tile([P, P], F32R, tag="mask_r")
            nc.vector.tensor_copy(mask_r, mask32)

            kT = kv_pool.tile([P, KT, S], F32R, tag="kT")      # K^T: [k_part, kt, s]
            v_sb = kv_pool.tile([P, ST, DV], F32R, tag="v")    # V: [s_part, st, v]
            qT = q_pool.tile([P, KT, S], F32R, tag="qT")       # Q^T: [k_part, kt, q]

            PSUM_BUFS = {"tp": 2, "mm": 4, "o": 2}

            def ps_tile(tag, w, dt):
                return psum.tile([P, w], dt, tag=tag, name=tag,
                                 bufs=PSUM_BUFS[tag])

            def ps_tile4(tag, dt):
                return psum.tile([P, 4, P], dt, tag=tag, name=tag,
                                 bufs=PSUM_BUFS[tag])

            # ---- Phase A: weight transposes -> wT[d_part, dt, k] ----
            with (
                tc.tile_pool(name="wkv", bufs=1) as wkv_pool,
                tc.tile_pool(name="wq", bufs=1) as wq_pool,
                tc.tile_pool(name="wstage", bufs=3) as wstage,
            ):
                wT = {
                    "wq": wq_pool.tile([P, DT, DK], F32R, tag="wqT", name="wqT"),
                    "wk": wkv_pool.tile([P, DT, DK], F32R, tag="wkT", name="wkT"),
                    "wv": wkv_pool.tile([P, DT, DK], F32R, tag="wvT", name="wvT"),
                }
                def emit_w_transposes():
                    for name in ("wk", "wv", "wq"):
                        for kt in range(KT):
                            wn = wstage.tile([P, D], F32, tag="wnat", bufs=2)
                            nc.gpsimd.dma_start(
                                out=wn,
                                in_=w_d[name][kt * P:(kt + 1) * P, :],
                            )
                            wr = wstage.tile([P, D], F32R, tag="wr", bufs=1)
                            nc.scalar.copy(wr, wn)
                            for a in range(DT // 4):
                                ps = ps_tile4("tp", F32R)
                                for j in range(4):
                                    dt_ = 4 * a + j
                                    nc.tensor.transpose(
                                        ps[:, j, :], wr[:, dt_ * P:(dt_ + 1) * P],
                                        ident_r,
                                    )
                                nc.vector.tensor_copy(
                                    wT[name][:, 4 * a:4 * a + 4, kt * P:(kt + 1) * P],
                                    ps,
                                )

                # ---- Phases B/C: X^T + projections, pipelined by quarters ----
                # (quarter = 512 rows = 4 s-tiles; transpose quarter t+1 on PE
                # overlaps DMA; projections of quarter t fill PE meanwhile)
                with tc.tile_pool(name="xstage", bufs=4) as xstage:
                    QS = 512           # quarter size in rows
                    QT4 = QS // P      # s-tiles per quarter

                    def emit_transpose_quarter(x_dram, qtr):
                        xT = xstage.tile([P, DT, QS], F32R, tag="xT", bufs=2)
                        for sl in range(QT4):
                            st = qtr * QT4 + sl
                            xn = xstage.tile([P, D], F32, tag="xnat", bufs=2)
                            eng = nc.sync if st % 2 == 0 else nc.gpsimd
                            eng.dma_start(
                                out=xn, in_=x_dram[st * P:(st + 1) * P, :]
                            )
                            xr = xstage.tile([P, D], F32R, tag="xr", bufs=2)
                            nc.scalar.copy(xr, xn)
                            for a in range(DT // 4):
                                ps = ps_tile4("tp", F32R)
                                for j in range(4):
                                    dt_ = 4 * a + j
                                    nc.tensor.transpose(
                                        ps[:, j, :], xr[:, dt_ * P:(dt_ + 1) * P],
                                        ident_r,
                                    )
                                nc.vector.tensor_copy(
                                    xT[:, 4 * a:4 * a + 4, sl * P:(sl + 1) * P], ps
                                )
                        return xT

                    def emit_proj_kv(qtr, xT):
                        for kt in range(KT):
                            ps = ps_tile("mm", 512, F32)
                            for dt_ in range(DT):
                                nc.tensor.matmul(
                                    ps,
                                    wT["wk"][:, dt_, kt * P:(kt + 1) * P],
                                    xT[:, dt_, :],
                                    start=(dt_ == 0), stop=(dt_ == DT - 1),
                                )
                            nc.scalar.copy(
                                kT[:, kt, qtr * QS:(qtr + 1) * QS], ps
                            )
                        for sl in range(QT4):
                            st = qtr * QT4 + sl
                            ps = ps_tile("mm", 512, F32)
                            for dt_ in range(DT):
                                nc.tensor.matmul(
                                    ps,
                                    xT[:, dt_, sl * P:(sl + 1) * P],
                                    wT["wv"][:, dt_, :],
                                    start=(dt_ == 0), stop=(dt_ == DT - 1),
                                )
                            nc.scalar.copy(v_sb[:, st, :], ps)

                    def emit_proj_q(qtr, xT):
                        for kt in range(KT):
                            ps = ps_tile("mm", 512, F32)
                            for dt_ in range(DT):
                                nc.tensor.matmul(
                                    ps,
                                    wT["wq"][:, dt_, kt * P:(kt + 1) * P],
                                    xT[:, dt_, :],
                                    start=(dt_ == 0), stop=(dt_ == DT - 1),
                                )
                            nc.scalar.mul(
                                qT[:, kt, qtr * QS:(qtr + 1) * QS], ps, SCALE
                            )

                    NQ = S // QS  # 4 quarters per tensor
                    stages = [("kv", q) for q in range(NQ)] + \
                             [("q", q) for q in range(NQ)]
                    prev = None
                    for idx, (kind, q) in enumerate(stages):
                        dram = xkv_d if kind == "kv" else xq_d
                        xT = emit_transpose_quarter(dram, q)
                        if idx == 0:
                            emit_w_transposes()
                        if prev is not None:
                            pk, pq, pxT = prev
                            (emit_proj_kv if pk == "kv" else emit_proj_q)(pq, pxT)
                        prev = (kind, q, xT)
                    pk, pq, pxT = prev
                    (emit_proj_kv if pk == "kv" else emit_proj_q)(pq, pxT)

            # ---- Phase D: attention, per q-tile ----
            with tc.tile_pool(name="attn", bufs=2) as attn:
                state = {}

                def emit_scores(i):
                    L = (i + 1) * P
                    NL = (L + 511) // 512
                    s_t = attn.tile([P, S], F32, tag="s_sb")
                    for c in range(NL):
                        w = min(512, L - c * 512)
                        ps = ps_tile("mm", 512, F32)
                        last_chunk = c == NL - 1
                        for kt in range(KT):
                            nc.tensor.matmul(
                                ps[:, :w],
                                qT[:, kt, i * P:(i + 1) * P],
                                kT[:, kt, c * 512:c * 512 + w],
                                start=(kt == 0),
                                stop=(kt == KT - 1 and not last_chunk),
                            )
                        if last_chunk:
                            # add causal mask to the diagonal 128 cols via PE
                            nc.tensor.matmul(
                                ps[:, w - P:w], ident_r, mask_r,
                                start=False, stop=True,
                            )
                        nc.scalar.copy(s_t[:, c * 512:c * 512 + w], ps[:, :w])
                    nmx = attn.tile([P, 1], F32, tag="nmx")
                    nc.vector.tensor_reduce(
                        out=nmx, in_=s_t[:, :L], axis=mybir.AxisListType.X,
                        op=mybir.AluOpType.max, negate=True,
                    )
                    p_t = attn.tile([P, S], F32R, tag="p_sb")
                    rs = attn.tile([P, 1], F32, tag="rs")
                    nc.scalar.activation(
                        out=p_t[:, :L], in_=s_t[:, :L],
                        func=mybir.ActivationFunctionType.Exp,
                        bias=nmx, scale=1.0, accum_out=rs,
                    )
                    rinv = attn.tile([P, 1], F32, tag="rinv")
                    nc.vector.reciprocal(rinv, rs)
                    state[i] = (p_t, rinv)

                def emit_out(i):
                    p_t, rinv = state.pop(i)
                    pT = attn.tile([P, ST, P], F32R, tag="pT")
                    for a in range((i + 4) // 4):
                        hi = min(4, i + 1 - 4 * a)
                        ps = ps_tile4("tp", F32R)
                        for j in range(hi):
                            st = 4 * a + j
                            nc.tensor.transpose(
                                ps[:, j, :], p_t[:, st * P:(st + 1) * P], ident_r
                            )
                        nc.vector.tensor_copy(
                            pT[:, 4 * a:4 * a + hi, :], ps[:, :hi, :]
                        )
                    ps_o = ps_tile("o", 512, F32)
                    for st in range(i + 1):
                        nc.tensor.matmul(
                            ps_o, pT[:, st, :], v_sb[:, st, :],
                            start=(st == 0), stop=(st == i),
                        )
                    o_t = attn.tile([P, DV], F32, tag="o_sb")
                    nc.scalar.activation(
                        out=o_t, in_=ps_o,
                        func=mybir.ActivationFunctionType.Copy, scale=rinv,
                    )
                    nc.sync.dma_start(out=out_d[i * P:(i + 1) * P, :], in_=o_t)

                for i in range(ST):
                    emit_scores(i)
                    if i > 0:
                        emit_out(i - 1)
                emit_out(ST - 1)

    nc.finalize()
    return nc


_NC = None


def _get_nc():
    global _NC
    if _NC is None:
        _NC = _build()
    return _NC


def kernel(source_query, source_key_value, source_query_padding_mask,
           source_key_value_padding_mask, Wq, Wk, Wv):
    nc = _get_nc()
    wq = np.ascontiguousarray(Wq, dtype=np.float32)
    wk = np.ascontiguousarray(Wk, dtype=np.float32)
    wv = np.ascontiguousarray(Wv, dtype=np.float32)
    in_maps = [
        {
            "xq": np.ascontiguousarray(source_query[c], dtype=np.float32),
            "xkv": np.ascontiguousarray(source_key_value[c], dtype=np.float32),
            "wq": wq, "wk": wk, "wv": wv,
        }
        for c in range(N_CORES)
    ]
    res = run_bass_kernel_spmd(nc, in_maps, list(range(N_CORES)))
    return np.stack([res.results[c]["out"] for c in range(N_CORES)]).astype(np.float32)


# revision 19
# speedup vs baseline: 303.6039x; 91.4828x over previous
"""Causal attention (B=8, S=2048, D=1024, d_k=d_v=512) on 8 TRN2 NeuronCores.

Sharding: data-parallel over batch — each core computes one batch element's
full attention. Weights are replicated. No collectives.

Per-core pipeline (all matmuls in float32r — full PE rate, ~1.5e-4 rel err):
  A) W^T tiles via PE transpose        [d, k] layout for projections
  B) Xkv^T via PE transpose -> K^T, V  (two halves to bound SBUF)
  C) Xq^T via PE transpose -> Q^T (scaled by 1/sqrt(d_k) at copyback)
  D) per q-tile i: S = Q^T.T @ K^T chunks into PSUM (causal: only s <= (i+1)*128),
     diagonal causal mask added via identity@mask matmul into PSUM,
     copy to SBUF (ACT), row-max (DVE, negated), exp+rowsum fused (ACT),
     P^T via PE transpose, O = P^T.T @ V accumulated in PSUM,
     O row-normalized by 1/rowsum at copyback, DMA out.
"""

import numpy as np

import concourse.bacc as bacc
import concourse.tile as tile
from concourse import mybir
from concourse.bass_utils import run_bass_kernel_spmd
from concourse.masks import make_identity

P = 128
S, D, DK, DV = 2048, 1024, 512, 512
ST, DT, KT = S // P, D // P, DK // P
SCALE = float(DK) ** -0.5
NEG = -30000.0
N_CORES = 8

F32 = mybir.dt.float32
F32R = mybir.dt.float32r


def _build():
    nc = bacc.Bacc(None, target_bir_lowering=False)
    xq_d = nc.declare_dram_parameter("xq", [S, D], F32, isOutput=False)
    xkv_d = nc.declare_dram_parameter("xkv", [S, D], F32, isOutput=False)
    w_d = {
        name: nc.declare_dram_parameter(name, [DK, D], F32, isOutput=False)
        for name in ("wq", "wk", "wv")
    }
    out_d = nc.declare_dram_parameter("out", [S, DV], F32, isOutput=True)

    with tile.TileContext(nc) as tc:
        with (
            tc.tile_pool(name="consts", bufs=1) as consts,
            tc.tile_pool(name="psum", bufs=1, space="PSUM") as psum,
            tc.tile_pool(name="kv", bufs=1) as kv_pool,
            tc.tile_pool(name="q", bufs=1) as q_pool,
        ):
            ident32 = consts.tile([P, P], F32, tag="ident32")
            make_identity(nc, ident32)
            ident_r = consts.tile([P, P], F32R, tag="ident_r")
            nc.vector.tensor_copy(ident_r, ident32)
            # causal mask for the diagonal block: 0 on/below diag, NEG above
            mask32 = consts.tile([P, P], F32, tag="mask32")
            nc.gpsimd.memset(mask32, 0.0)
            nc.gpsimd.affine_select(
                out=mask32, in_=mask32, compare_op=mybir.AluOpType.is_ge,
                fill=NEG, base=0, pattern=[[-1, P]], channel_multiplier=1,
            )
            mask_bf = consts.tile([P, P], mybir.dt.bfloat16, tag="mask_bf")
            nc.vector.tensor_copy(mask_bf, mask32)
            ident_bf = consts.tile([P, P], mybir.dt.bfloat16, tag="ident_bf")
            nc.vector.tensor_copy(ident_bf, ident32)

            kT = kv_pool.tile([P, KT, S], F32R, tag="kT")      # K^T: [k_part, kt, s]
            v_sb = kv_pool.tile([P, ST, DV], F32R, tag="v")    # V: [s_part, st, v]
            qT = q_pool.tile([P, KT, S], F32R, tag="qT")       # Q^T: [k_part, kt, q]

            PSUM_BUFS = {"tp": 2, "mm": 4, "o": 2}

            def ps_tile(tag, w, dt):
                return psum.tile([P, w], dt, tag=tag, name=tag,
                                 bufs=PSUM_BUFS[tag])

            def ps_tile4(tag, dt):
                return psum.tile([P, 4, P], dt, tag=tag, name=tag,
                                 bufs=PSUM_BUFS[tag])

            # ---- Phase A: weight transposes -> wT[d_part, dt, k] ----
            with (
                tc.tile_pool(name="wkv", bufs=1) as wkv_pool,
                tc.tile_pool(name="wq", bufs=1) as wq_pool,
                tc.tile_pool(name="wstage", bufs=3) as wstage,
            ):
                wT = {
                    "wq": wq_pool.tile([P, DT, DK], F32R, tag="wqT", name="wqT"),
                    "wk": wkv_pool.tile([P, DT, DK], F32R, tag="wkT", name="wkT"),
                    "wv": wkv_pool.tile([P, DT, DK], F32R, tag="wvT", name="wvT"),
                }
                def emit_w_transposes():
                    for name in ("wk", "wv", "wq"):
                        for kt in range(KT):
                            wn = wstage.tile([P, D], F32, tag="wnat", bufs=2)
                            nc.gpsimd.dma_start(
                                out=wn,
                                in_=w_d[name][kt * P:(kt + 1) * P, :],
                            )
                            wr = wstage.tile([P, D], F32R, tag="wr", bufs=1)
                            nc.scalar.copy(wr, wn)
                            for a in range(DT // 4):
                                ps = ps_tile4("tp", F32R)
                                for j in range(4):
                                    dt_ = 4 * a + j
                                    nc.tensor.transpose(
                                        ps[:, j, :], wr[:, dt_ * P:(dt_ + 1) * P],
                                        ident_r,
                                    )
                                nc.vector.tensor_copy(
                                    wT[name][:, 4 * a:4 * a + 4, kt * P:(kt + 1) * P],
                                    ps,
                                )

                # ---- Phases B/C: X^T + projections, pipelined by quarters ----
                # (quarter = 512 rows = 4 s-tiles; transpose quarter t+1 on PE
                # overlaps DMA; projections of quarter t fill PE meanwhile)
                with tc.tile_pool(name="xstage", bufs=4) as xstage:
                    QS = 512           # quarter size in rows
                    QT4 = QS // P      # s-tiles per quarter

                    def emit_transpose_quarter(x_dram, qtr):
                        xT = xstage.tile([P, DT, QS], F32R, tag="xT", bufs=2)
                        for sl in range(QT4):
                            st = qtr * QT4 + sl
                            xn = xstage.tile([P, D], F32, tag="xnat", bufs=2)
                            eng = nc.sync if st % 2 == 0 else nc.gpsimd
                            eng.dma_start(
                                out=xn, in_=x_dram[st * P:(st + 1) * P, :]
                            )
                            xr = xstage.tile([P, D], F32R, tag="xr", bufs=2)
                            nc.scalar.copy(xr, xn)
                            for a in range(DT // 4):
                                ps = ps_tile4("tp", F32R)
                                for j in range(4):
                                    dt_ = 4 * a + j
                                    nc.tensor.transpose(
                                        ps[:, j, :], xr[:, dt_ * P:(dt_ + 1) * P],
                                        ident_r,
                                    )
                                nc.vector.tensor_copy(
                                    xT[:, 4 * a:4 * a + 4, sl * P:(sl + 1) * P], ps
                                )
                        return xT

                    def emit_proj_kv(qtr, xT):
                        for kt in range(KT):
                            ps = ps_tile("mm", 512, F32)
                            for dt_ in range(DT):
                                nc.tensor.matmul(
                                    ps,
                                    wT["wk"][:, dt_, kt * P:(kt + 1) * P],
                                    xT[:, dt_, :],
                                    start=(dt_ == 0), stop=(dt_ == DT - 1),
                                )
                            nc.scalar.copy(
                                kT[:, kt, qtr * QS:(qtr + 1) * QS], ps
                            )
                        for sl in range(QT4):
                            st = qtr * QT4 + sl
                            ps = ps_tile("mm", 512, F32)
                            for dt_ in range(DT):
                                nc.tensor.matmul(
                                    ps,
                                    xT[:, dt_, sl * P:(sl + 1) * P],
                                    wT["wv"][:, dt_, :],
                                    start=(dt_ == 0), stop=(dt_ == DT - 1),
                                )
                            nc.scalar.copy(v_sb[:, st, :], ps)

                    def emit_proj_q(qtr, xT):
                        for kt in range(KT):
                            ps = ps_tile("mm", 512, F32)
                            for dt_ in range(DT):
                                nc.tensor.matmul(
                                    ps,
                                    wT["wq"][:, dt_, kt * P:(kt + 1) * P],
                                    xT[:, dt_, :],
                                    start=(dt_ == 0), stop=(dt_ == DT - 1),
                                )
                            nc.scalar.mul(
                                qT[:, kt, qtr * QS:(qtr + 1) * QS], ps, SCALE
                            )

                    NQ = S // QS  # 4 quarters per tensor
                    stages = [("kv", q) for q in range(NQ)] + \
                             [("q", q) for q in range(NQ)]
                    prev = None
                    for idx, (kind, q) in enumerate(stages):
                        dram = xkv_d if kind == "kv" else xq_d
                        xT = emit_transpose_quarter(dram, q)
                        if idx == 0:
                            emit_w_transposes()
                        if prev is not None:
                            pk, pq, pxT = prev
                            (emit_proj_kv if pk == "kv" else emit_proj_q)(pq, pxT)
                        prev = (kind, q, xT)
                    pk, pq, pxT = prev
                    (emit_proj_kv if pk == "kv" else emit_proj_q)(pq, pxT)

            # ---- Phase D: attention, per q-tile ----
            with tc.tile_pool(name="attn", bufs=3) as attn:
                state = {}

                def emit_scores(i):
                    L = (i + 1) * P
                    # chunk widths: keep every chunk >= 256 (fp32r runs
                    # 4 cyc/row below 256) except the unavoidable L=128 case
                    widths = []
                    rem = L
                    while rem > 640:
                        widths.append(512)
                        rem -= 512
                    if rem == 640:
                        widths += [384, 256]
                    else:
                        widths.append(rem)  # 128, 256, 384 or 512
                    s_t = attn.tile([P, S], F32, tag="s_sb")
                    off = 0
                    for c, w in enumerate(widths):
                        ps = ps_tile("mm", 512, F32)
                        last_chunk = c == len(widths) - 1
                        for kt in range(KT):
                            nc.tensor.matmul(
                                ps[:, :w],
                                qT[:, kt, i * P:(i + 1) * P],
                                kT[:, kt, off:off + w],
                                start=(kt == 0),
                                stop=(kt == KT - 1 and not last_chunk),
                            )
                        if last_chunk:
                            # add causal mask to the diagonal 128 cols via PE
                            nc.tensor.matmul(
                                ps[:, w - P:w], ident_bf, mask_bf,
                                start=False, stop=True,
                            )
                        nc.scalar.copy(s_t[:, off:off + w], ps[:, :w])
                        off += w
                    nmx = attn.tile([P, 1], F32, tag="nmx")
                    nc.vector.tensor_reduce(
                        out=nmx, in_=s_t[:, :L], axis=mybir.AxisListType.X,
                        op=mybir.AluOpType.max, negate=True,
                    )
                    p_t = attn.tile([P, S], F32R, tag="p_sb")
                    rs = attn.tile([P, 1], F32, tag="rs")
                    nc.scalar.activation(
                        out=p_t[:, :L], in_=s_t[:, :L],
                        func=mybir.ActivationFunctionType.Exp,
                        bias=nmx, scale=1.0, accum_out=rs,
                    )
                    rinv = attn.tile([P, 1], F32, tag="rinv")
                    nc.vector.reciprocal(rinv, rs)
                    state[i] = (p_t, rinv)

                def emit_out(i):
                    p_t, rinv = state.pop(i)
                    pT = attn.tile([P, ST, P], F32R, tag="pT")
                    for a in range((i + 4) // 4):
                        hi = min(4, i + 1 - 4 * a)
                        ps = ps_tile4("tp", F32R)
                        for j in range(hi):
                            st = 4 * a + j
                            nc.tensor.transpose(
                                ps[:, j, :], p_t[:, st * P:(st + 1) * P], ident_r
                            )
                        nc.vector.tensor_copy(
                            pT[:, 4 * a:4 * a + hi, :], ps[:, :hi, :]
                        )
                    ps_o = ps_tile("o", 512, F32)
                    for st in range(i + 1):
                        nc.tensor.matmul(
                            ps_o, pT[:, st, :], v_sb[:, st, :],
                            start=(st == 0), stop=(st == i),
                        )
                    o_t = attn.tile([P, DV], F32, tag="o_sb")
                    nc.scalar.activation(
                        out=o_t, in_=ps_o,
                        func=mybir.ActivationFunctionType.Copy, scale=rinv,
                    )
                    nc.sync.dma_start(out=out_d[i * P:(i + 1) * P, :], in_=o_t)

                LOOKAHEAD = 2
                for i in range(ST):
                    emit_scores(i)
                    if i >= LOOKAHEAD:
                        emit_out(i - LOOKAHEAD)
                for i in range(ST - LOOKAHEAD, ST):
                    emit_out(i)

    nc.finalize()
    return nc


_NC = None


def _get_nc():
    global _NC
    if _NC is None:
        _NC = _build()
    return _NC


def kernel(source_query, source_key_value, source_query_padding_mask,
           source_key_value_padding_mask, Wq, Wk, Wv):
    nc = _get_nc()
    wq = np.ascontiguousarray(Wq, dtype=np.float32)
    wk = np.ascontiguousarray(Wk, dtype=np.float32)
    wv = np.ascontiguousarray(Wv, dtype=np.float32)
    in_maps = [
        {
            "xq": np.ascontiguousarray(source_query[c], dtype=np.float32),
            "xkv": np.ascontiguousarray(source_key_value[c], dtype=np.float32),
            "wq": wq, "wk": wk, "wv": wv,
        }
        for c in range(N_CORES)
    ]
    res = run_bass_kernel_spmd(nc, in_maps, list(range(N_CORES)))
    return np.stack([res.results[c]["out"] for c in range(N_CORES)]).astype(np.float32)


# revision 37
# speedup vs baseline: 316.2807x; 1.0418x over previous
"""Causal attention (B=8, S=2048, D=1024, d_k=d_v=512) on 8 TRN2 NeuronCores.

Sharding: data-parallel over batch — each core computes one batch element's
full attention. Weights are replicated. No collectives. The padding masks are
all-False by construction (spec fill=zeros), so only causal masking applies.

Per-core pipeline (all matmuls in float32r — full PE rate, ~1.5e-4 rel err):
  - X^T / W^T via PE transposes (fp32r, 4 per PSUM bank, one batched DVE
    copyback each); inputs DMA'd in 512-col halves on two DMA engines and
    rounded to fp32r on ACT.
  - Projections pipelined with transposes at 512-row-quarter granularity:
    Q^T/K^T as [d_k, seq] (1/sqrt(d_k) folded into Q^T copyback), V as [s, v].
  - Per q-tile i (128 rows): S chunks (all >=256 wide) accumulate in PSUM over
    4 k-tiles; causal: only s <= (i+1)*128 computed; diagonal block masked by
    a bf16 identity@mask matmul adding -30000 into PSUM; chunks copied to SBUF
    (ACT); row-max (DVE tensor_reduce negate); exp + row-sum fused in one ACT
    pass (accum_out); P^T via PE transpose; O = P^T.T @ V accumulated in
    PSUM; O scaled by 1/rowsum (ACT Copy, scale=AP) and DMA'd out.
  - Phase D runs with lookahead 2: scores(i+1), scores(i+2) are emitted before
    out(i) so PE stays busy during softmax latency.
"""

import numpy as np

import concourse.bacc as bacc
import concourse.tile as tile
from concourse import mybir
from concourse.bass_utils import run_bass_kernel_spmd
from concourse.masks import make_identity

P = 128
S, D, DK, DV = 2048, 1024, 512, 512
ST, DT, KT = S // P, D // P, DK // P
SCALE = float(DK) ** -0.5
NEG = -30000.0
N_CORES = 8

F32 = mybir.dt.float32
F32R = mybir.dt.float32r


def _build():
    nc = bacc.Bacc(None, target_bir_lowering=False)
    xq_d = nc.declare_dram_parameter("xq", [S, D], F32, isOutput=False)
    xkv_d = nc.declare_dram_parameter("xkv", [S, D], F32, isOutput=False)
    w_d = {
        name: nc.declare_dram_parameter(name, [DK, D], F32, isOutput=False)
        for name in ("wq", "wk", "wv")
    }
    out_d = nc.declare_dram_parameter("out", [S, DV], F32, isOutput=True)

    with tile.TileContext(nc) as tc:
        with (
            tc.tile_pool(name="consts", bufs=1) as consts,
            tc.tile_pool(name="psum", bufs=1, space="PSUM") as psum,
            tc.tile_pool(name="kv", bufs=1) as kv_pool,
            tc.tile_pool(name="q", bufs=1) as q_pool,
        ):
            ident32 = consts.tile([P, P], F32, tag="ident32")
            make_identity(nc, ident32)
            ident_r = consts.tile([P, P], F32R, tag="ident_r")
            nc.vector.tensor_copy(ident_r, ident32)
            # causal mask for the diagonal block: 0 on/below diag, NEG above
            mask32 = consts.tile([P, P], F32, tag="mask32")
            nc.gpsimd.memset(mask32, 0.0)
            nc.gpsimd.affine_select(
                out=mask32, in_=mask32, compare_op=mybir.AluOpType.is_ge,
                fill=NEG, base=0, pattern=[[-1, P]], channel_multiplier=1,
            )
            mask_bf = consts.tile([P, P], mybir.dt.bfloat16, tag="mask_bf")
            nc.vector.tensor_copy(mask_bf, mask32)
            ident_bf = consts.tile([P, P], mybir.dt.bfloat16, tag="ident_bf")
            nc.vector.tensor_copy(ident_bf, ident32)

            kT = kv_pool.tile([P, KT, S], F32R, tag="kT")      # K^T: [k_part, kt, s]
            v_sb = kv_pool.tile([P, ST, DV], F32R, tag="v")    # V: [s_part, st, v]
            qT = q_pool.tile([P, KT, S], F32R, tag="qT")       # Q^T: [k_part, kt, q]

            PSUM_BUFS = {"tp": 4, "mm": 3, "o": 1}

            def ps_tile(tag, w, dt):
                return psum.tile([P, w], dt, tag=tag, name=tag,
                                 bufs=PSUM_BUFS[tag])

            def ps_tile4(tag, dt):
                return psum.tile([P, 4, P], dt, tag=tag, name=tag,
                                 bufs=PSUM_BUFS[tag])

            # ---- Phase A: weight transposes -> wT[d_part, dt, k] ----
            with (
                tc.tile_pool(name="wkv", bufs=1) as wkv_pool,
                tc.tile_pool(name="wq", bufs=1) as wq_pool,
                tc.tile_pool(name="wstage", bufs=3) as wstage,
            ):
                wT = {
                    "wq": wq_pool.tile([P, DT, DK], F32R, tag="wqT", name="wqT"),
                    "wk": wkv_pool.tile([P, DT, DK], F32R, tag="wkT", name="wkT"),
                    "wv": wkv_pool.tile([P, DT, DK], F32R, tag="wvT", name="wvT"),
                }
                def emit_w_transposes(names):
                    for name in names:
                        for kt in range(KT):
                            for a in range(DT // 4):
                                wn = wstage.tile([P, D // 2], F32, tag="wnat",
                                                 bufs=3, name="wn")
                                weng = nc.gpsimd if (kt + a) % 2 == 0 else nc.sync
                                weng.dma_start(
                                    out=wn,
                                    in_=w_d[name][kt * P:(kt + 1) * P,
                                                  a * 512:(a + 1) * 512],
                                )
                                wr = wstage.tile([P, D // 2], F32R, tag="wr",
                                                 bufs=2, name="wr")
                                nc.scalar.copy(wr, wn)
                                ps = ps_tile4("tp", F32R)
                                for j in range(4):
                                    nc.tensor.transpose(
                                        ps[:, j, :], wr[:, j * P:(j + 1) * P],
                                        ident_r,
                                    )
                                nc.vector.tensor_copy(
                                    wT[name][:, 4 * a:4 * a + 4, kt * P:(kt + 1) * P],
                                    ps,
                                )

                # ---- Phases B/C: X^T + projections, pipelined by quarters ----
                # (quarter = 512 rows = 4 s-tiles; transpose quarter t+1 on PE
                # overlaps DMA; projections of quarter t fill PE meanwhile)
                with tc.tile_pool(name="xstage", bufs=4) as xstage:
                    QS = 512           # quarter size in rows
                    QT4 = QS // P      # s-tiles per quarter

                    def emit_transpose_quarter(x_dram, qtr):
                        xT = xstage.tile([P, DT, QS], F32R, tag="xT", bufs=2)
                        for sl in range(QT4):
                            st = qtr * QT4 + sl
                            # load + round in 512-col halves on both DMA
                            # engines: halves the DMA->round->transpose chain
                            for a in range(DT // 4):
                                xn = xstage.tile([P, D // 2], F32, tag="xnat",
                                                 bufs=4, name="xn")
                                eng = nc.sync if (2 * st + a) % 2 == 0 else nc.gpsimd
                                eng.dma_start(
                                    out=xn,
                                    in_=x_dram[st * P:(st + 1) * P,
                                               a * 512:(a + 1) * 512],
                                )
                                xr = xstage.tile([P, D // 2], F32R, tag="xr",
                                                 bufs=4, name="xr")
                                nc.scalar.copy(xr, xn)
                                ps = ps_tile4("tp", F32R)
                                for j in range(4):
                                    nc.tensor.transpose(
                                        ps[:, j, :], xr[:, j * P:(j + 1) * P],
                                        ident_r,
                                    )
                                nc.vector.tensor_copy(
                                    xT[:, 4 * a:4 * a + 4, sl * P:(sl + 1) * P], ps
                                )
                        return xT

                    def emit_proj_kv(qtr, xT):
                        for kt in range(KT):
                            ps = ps_tile("mm", 512, F32)
                            for dt_ in range(DT):
                                nc.tensor.matmul(
                                    ps,
                                    wT["wk"][:, dt_, kt * P:(kt + 1) * P],
                                    xT[:, dt_, :],
                                    start=(dt_ == 0), stop=(dt_ == DT - 1),
                                )
                            nc.scalar.copy(
                                kT[:, kt, qtr * QS:(qtr + 1) * QS], ps
                            )
                        for sl in range(QT4):
                            st = qtr * QT4 + sl
                            ps = ps_tile("mm", 512, F32)
                            for dt_ in range(DT):
                                nc.tensor.matmul(
                                    ps,
                                    xT[:, dt_, sl * P:(sl + 1) * P],
                                    wT["wv"][:, dt_, :],
                                    start=(dt_ == 0), stop=(dt_ == DT - 1),
                                )
                            nc.vector.tensor_copy(v_sb[:, st, :], ps)

                    def emit_proj_q(qtr, xT):
                        for kt in range(KT):
                            ps = ps_tile("mm", 512, F32)
                            for dt_ in range(DT):
                                nc.tensor.matmul(
                                    ps,
                                    wT["wq"][:, dt_, kt * P:(kt + 1) * P],
                                    xT[:, dt_, :],
                                    start=(dt_ == 0), stop=(dt_ == DT - 1),
                                )
                            nc.scalar.mul(
                                qT[:, kt, qtr * QS:(qtr + 1) * QS], ps, SCALE
                            )

                    NQ = S // QS  # 4 quarters per tensor
                    stages = [("kv", q) for q in range(NQ)] + \
                             [("q", q) for q in range(NQ)]
                    prev = None
                    for idx, (kind, q) in enumerate(stages):
                        dram = xkv_d if kind == "kv" else xq_d
                        xT = emit_transpose_quarter(dram, q)
                        if idx == 0:
                            # wk/wv needed for proj(kv0); wq much later --
                            # emitting it early would stall PE on wq DMAs
                            emit_w_transposes(("wk", "wv"))
                        if prev is not None:
                            pk, pq, pxT = prev
                            (emit_proj_kv if pk == "kv" else emit_proj_q)(pq, pxT)
                        if idx == 2:
                            # after proj(kv1): wq DMAs have had time to land
                            emit_w_transposes(("wq",))
                        prev = (kind, q, xT)
                    pk, pq, pxT = prev
                    (emit_proj_kv if pk == "kv" else emit_proj_q)(pq, pxT)

            # ---- Phase D: attention, per q-tile ----
            with tc.tile_pool(name="attn", bufs=3) as attn:
                state = {}

                def emit_scores(i):
                    L = (i + 1) * P
                    # chunk widths: keep every chunk >= 256 (fp32r runs
                    # 4 cyc/row below 256) except the unavoidable L=128 case
                    widths = []
                    rem = L
                    while rem > 640:
                        widths.append(512)
                        rem -= 512
                    if rem == 640:
                        widths += [384, 256]
                    else:
                        widths.append(rem)  # 128, 256, 384 or 512
                    s_t = attn.tile([P, S], F32, tag="s_sb")
                    off = 0
                    for c, w in enumerate(widths):
                        ps = ps_tile("mm", 512, F32)
                        last_chunk = c == len(widths) - 1
                        for kt in range(KT):
                            nc.tensor.matmul(
                                ps[:, :w],
                                qT[:, kt, i * P:(i + 1) * P],
                                kT[:, kt, off:off + w],
                                start=(kt == 0),
                                stop=(kt == KT - 1 and not last_chunk),
                            )
                        if last_chunk:
                            # add causal mask to the diagonal 128 cols via PE
                            nc.tensor.matmul(
                                ps[:, w - P:w], ident_bf, mask_bf,
                                start=False, stop=True,
                            )
                        nc.scalar.copy(s_t[:, off:off + w], ps[:, :w])
                        off += w
                    nmx = attn.tile([P, 1], F32, tag="nmx")
                    nc.vector.tensor_reduce(
                        out=nmx, in_=s_t[:, :L], axis=mybir.AxisListType.X,
                        op=mybir.AluOpType.max, negate=True,
                    )
                    p_t = attn.tile([P, S], F32R, tag="p_sb")
                    rs = attn.tile([P, 1], F32, tag="rs")
                    nc.scalar.activation(
                        out=p_t[:, :L], in_=s_t[:, :L],
                        func=mybir.ActivationFunctionType.Exp,
                        bias=nmx, scale=1.0, accum_out=rs,
                    )
                    rinv = attn.tile([P, 1], F32, tag="rinv")
                    nc.vector.reciprocal(rinv, rs)
                    state[i] = (p_t, rinv)

                def emit_out(i):
                    p_t, rinv = state.pop(i)
                    pT = attn.tile([P, ST, P], F32R, tag="pT")
                    for a in range((i + 4) // 4):
                        hi = min(4, i + 1 - 4 * a)
                        ps = ps_tile4("tp", F32R)
                        for j in range(hi):
                            st = 4 * a + j
                            nc.tensor.transpose(
                                ps[:, j, :], p_t[:, st * P:(st + 1) * P], ident_r
                            )
                        nc.vector.tensor_copy(
                            pT[:, 4 * a:4 * a + hi, :], ps[:, :hi, :]
                        )
                    ps_o = ps_tile("o", 512, F32)
                    for st in range(i + 1):
                        nc.tensor.matmul(
                            ps_o, pT[:, st, :], v_sb[:, st, :],
                            start=(st == 0), stop=(st == i),
                        )
                    o_t = attn.tile([P, DV], F32, tag="o_sb")
                    nc.scalar.activation(
                        out=o_t, in_=ps_o,
                        func=mybir.ActivationFunctionType.Copy, scale=rinv,
                    )
                    nc.sync.dma_start(out=out_d[i * P:(i + 1) * P, :], in_=o_t)

                LOOKAHEAD = 2
                for i in range(ST):
                    emit_scores(i)
                    if i >= LOOKAHEAD:
                        emit_out(i - LOOKAHEAD)
                for i in range(ST - LOOKAHEAD, ST):
                    emit_out(i)

    nc.finalize()
    return nc


_NC = None


def _get_nc():
    global _NC
    if _NC is None:
        _NC = _build()
    return _NC


def kernel(source_query, source_key_value, source_query_padding_mask,
           source_key_value_padding_mask, Wq, Wk, Wv):
    nc = _get_nc()
    wq = np.ascontiguousarray(Wq, dtype=np.float32)
    wk = np.ascontiguousarray(Wk, dtype=np.float32)
    wv = np.ascontiguousarray(Wv, dtype=np.float32)
    in_maps = [
        {
            "xq": np.ascontiguousarray(source_query[c], dtype=np.float32),
            "xkv": np.ascontiguousarray(source_key_value[c], dtype=np.float32),
            "wq": wq, "wk": wk, "wv": wv,
        }
        for c in range(N_CORES)
    ]
    try:
        res = run_bass_kernel_spmd(nc, in_maps, list(range(N_CORES)))
    except Exception:
        # transient NRT device errors have been observed through the axon
        # tunnel; one retry is usually enough
        res = run_bass_kernel_spmd(nc, in_maps, list(range(N_CORES)))
    return np.stack([res.results[c]["out"] for c in range(N_CORES)]).astype(np.float32)


# revision 42
# speedup vs baseline: 317.9812x; 1.0054x over previous
"""Causal attention (B=8, S=2048, D=1024, d_k=d_v=512) on 8 TRN2 NeuronCores.

Sharding: data-parallel over batch — each core computes one batch element's
full attention. Weights are replicated. No collectives. The padding masks are
all-False by construction (spec fill=zeros), so only causal masking applies.

Per-core pipeline (all matmuls in float32r — full PE rate, ~1.5e-4 rel err):
  - X^T / W^T via PE transposes (fp32r, 4 per PSUM bank, one batched DVE
    copyback each); inputs DMA'd in 512-col halves on two DMA engines and
    rounded to fp32r on ACT.
  - Projections pipelined with transposes at 512-row-quarter granularity:
    Q^T/K^T as [d_k, seq] (1/sqrt(d_k) folded into Q^T copyback), V as [s, v].
  - Per q-tile i (128 rows): S chunks (all >=256 wide) accumulate in PSUM over
    4 k-tiles; causal: only s <= (i+1)*128 computed; diagonal block masked by
    a bf16 identity@mask matmul adding -30000 into PSUM; chunks copied to SBUF
    (ACT); row-max (DVE tensor_reduce negate); exp + row-sum fused in one ACT
    pass (accum_out); P^T via PE transpose; O = P^T.T @ V accumulated in
    PSUM; O scaled by 1/rowsum (ACT Copy, scale=AP) and DMA'd out.
  - Phase D runs with lookahead 2: scores(i+1), scores(i+2) are emitted before
    out(i) so PE stays busy during softmax latency.
"""

import numpy as np

import concourse.bacc as bacc
import concourse.tile as tile
from concourse import mybir
from concourse.bass_utils import run_bass_kernel_spmd
from concourse.masks import make_identity

P = 128
S, D, DK, DV = 2048, 1024, 512, 512
ST, DT, KT = S // P, D // P, DK // P
SCALE = float(DK) ** -0.5
NEG = -30000.0
N_CORES = 8

F32 = mybir.dt.float32
F32R = mybir.dt.float32r


def _build():
    nc = bacc.Bacc(None, target_bir_lowering=False)
    xq_d = nc.declare_dram_parameter("xq", [S, D], F32, isOutput=False)
    xkv_d = nc.declare_dram_parameter("xkv", [S, D], F32, isOutput=False)
    w_d = {
        name: nc.declare_dram_parameter(name, [DK, D], F32, isOutput=False)
        for name in ("wq", "wk", "wv")
    }
    out_d = nc.declare_dram_parameter("out", [S, DV], F32, isOutput=True)

    with tile.TileContext(nc) as tc:
        with (
            tc.tile_pool(name="consts", bufs=1) as consts,
            tc.tile_pool(name="psum", bufs=1, space="PSUM") as psum,
            tc.tile_pool(name="kv", bufs=1) as kv_pool,
            tc.tile_pool(name="q", bufs=1) as q_pool,
        ):
            ident32 = consts.tile([P, P], F32, tag="ident32")
            make_identity(nc, ident32)
            ident_r = consts.tile([P, P], F32R, tag="ident_r")
            nc.vector.tensor_copy(ident_r, ident32)
            # causal mask for the diagonal block: 0 on/below diag, NEG above
            mask32 = consts.tile([P, P], F32, tag="mask32")
            nc.gpsimd.memset(mask32, 0.0)
            nc.gpsimd.affine_select(
                out=mask32, in_=mask32, compare_op=mybir.AluOpType.is_ge,
                fill=NEG, base=0, pattern=[[-1, P]], channel_multiplier=1,
            )
            mask_bf = consts.tile([P, P], mybir.dt.bfloat16, tag="mask_bf")
            nc.vector.tensor_copy(mask_bf, mask32)
            ident_bf = consts.tile([P, P], mybir.dt.bfloat16, tag="ident_bf")
            nc.vector.tensor_copy(ident_bf, ident32)

            kT = kv_pool.tile([P, KT, S], F32R, tag="kT")      # K^T: [k_part, kt, s]
            v_sb = kv_pool.tile([P, ST, DV], F32R, tag="v")    # V: [s_part, st, v]
            qT = q_pool.tile([P, KT, S], F32R, tag="qT")       # Q^T: [k_part, kt, q]

            PSUM_BUFS = {"tp": 4, "mm": 3, "o": 1}

            def ps_tile(tag, w, dt):
                return psum.tile([P, w], dt, tag=tag, name=tag,
                                 bufs=PSUM_BUFS[tag])

            def ps_tile4(tag, dt):
                return psum.tile([P, 4, P], dt, tag=tag, name=tag,
                                 bufs=PSUM_BUFS[tag])

            # ---- Phase A: weight transposes -> wT[d_part, dt, k] ----
            with (
                tc.tile_pool(name="wkv", bufs=1) as wkv_pool,
                tc.tile_pool(name="wq", bufs=1) as wq_pool,
                tc.tile_pool(name="wstage", bufs=3) as wstage,
            ):
                wT = {
                    "wq": wq_pool.tile([P, DT, DK], F32R, tag="wqT", name="wqT"),
                    "wk": wkv_pool.tile([P, DT, DK], F32R, tag="wkT", name="wkT"),
                    "wv": wkv_pool.tile([P, DT, DK], F32R, tag="wvT", name="wvT"),
                }
                def emit_w_transposes(names):
                    for name in names:
                        for kt in range(KT):
                            for a in range(DT // 4):
                                wn = wstage.tile([P, D // 2], F32, tag="wnat",
                                                 bufs=3, name="wn")
                                weng = nc.gpsimd if (kt + a) % 2 == 0 else nc.sync
                                weng.dma_start(
                                    out=wn,
                                    in_=w_d[name][kt * P:(kt + 1) * P,
                                                  a * 512:(a + 1) * 512],
                                )
                                wr = wstage.tile([P, D // 2], F32R, tag="wr",
                                                 bufs=2, name="wr")
                                nc.scalar.copy(wr, wn)
                                ps = ps_tile4("tp", F32R)
                                for j in range(4):
                                    nc.tensor.transpose(
                                        ps[:, j, :], wr[:, j * P:(j + 1) * P],
                                        ident_r,
                                    )
                                nc.vector.tensor_copy(
                                    wT[name][:, 4 * a:4 * a + 4, kt * P:(kt + 1) * P],
                                    ps,
                                )

                # ---- Phases B/C: X^T + projections, pipelined by quarters ----
                # (quarter = 512 rows = 4 s-tiles; transpose quarter t+1 on PE
                # overlaps DMA; projections of quarter t fill PE meanwhile)
                with tc.tile_pool(name="xstage", bufs=4) as xstage:
                    QS = 512           # quarter size in rows
                    QT4 = QS // P      # s-tiles per quarter

                    def emit_transpose_quarter(x_dram, qtr):
                        xT = xstage.tile([P, DT, QS], F32R, tag="xT", bufs=2)
                        for sl in range(QT4):
                            st = qtr * QT4 + sl
                            # load + round in 512-col halves on both DMA
                            # engines: halves the DMA->round->transpose chain
                            for a in range(DT // 4):
                                xn = xstage.tile([P, D // 2], F32, tag="xnat",
                                                 bufs=4, name="xn")
                                eng = nc.sync if (2 * st + a) % 2 == 0 else nc.gpsimd
                                eng.dma_start(
                                    out=xn,
                                    in_=x_dram[st * P:(st + 1) * P,
                                               a * 512:(a + 1) * 512],
                                )
                                xr = xstage.tile([P, D // 2], F32R, tag="xr",
                                                 bufs=4, name="xr")
                                nc.scalar.copy(xr, xn)
                                ps = ps_tile4("tp", F32R)
                                for j in range(4):
                                    nc.tensor.transpose(
                                        ps[:, j, :], xr[:, j * P:(j + 1) * P],
                                        ident_r,
                                    )
                                nc.vector.tensor_copy(
                                    xT[:, 4 * a:4 * a + 4, sl * P:(sl + 1) * P], ps
                                )
                        return xT

                    def emit_proj_kv(qtr, xT):
                        for kt in range(KT):
                            ps = ps_tile("mm", 512, F32)
                            for dt_ in range(DT):
                                nc.tensor.matmul(
                                    ps,
                                    wT["wk"][:, dt_, kt * P:(kt + 1) * P],
                                    xT[:, dt_, :],
                                    start=(dt_ == 0), stop=(dt_ == DT - 1),
                                )
                            nc.vector.tensor_copy(
                                kT[:, kt, qtr * QS:(qtr + 1) * QS], ps
                            )
                        for sl in range(QT4):
                            st = qtr * QT4 + sl
                            ps = ps_tile("mm", 512, F32)
                            for dt_ in range(DT):
                                nc.tensor.matmul(
                                    ps,
                                    xT[:, dt_, sl * P:(sl + 1) * P],
                                    wT["wv"][:, dt_, :],
                                    start=(dt_ == 0), stop=(dt_ == DT - 1),
                                )
                            nc.vector.tensor_copy(v_sb[:, st, :], ps)

                    def emit_proj_q(qtr, xT):
                        for kt in range(KT):
                            ps = ps_tile("mm", 512, F32)
                            for dt_ in range(DT):
                                nc.tensor.matmul(
                                    ps,
                                    wT["wq"][:, dt_, kt * P:(kt + 1) * P],
                                    xT[:, dt_, :],
                                    start=(dt_ == 0), stop=(dt_ == DT - 1),
                                )
                            nc.vector.tensor_scalar_mul(
                                qT[:, kt, qtr * QS:(qtr + 1) * QS], ps, SCALE
                            )

                    NQ = S // QS  # 4 quarters per tensor
                    stages = [("kv", q) for q in range(NQ)] + \
                             [("q", q) for q in range(NQ)]
                    prev = None
                    for idx, (kind, q) in enumerate(stages):
                        dram = xkv_d if kind == "kv" else xq_d
                        xT = emit_transpose_quarter(dram, q)
                        if idx == 0:
                            # wk/wv needed for proj(kv0); wq much later --
                            # emitting it early would stall PE on wq DMAs
                            emit_w_transposes(("wk", "wv"))
                        if prev is not None:
                            pk, pq, pxT = prev
                            (emit_proj_kv if pk == "kv" else emit_proj_q)(pq, pxT)
                        if idx == 2:
                            # after proj(kv1): wq DMAs have had time to land
                            emit_w_transposes(("wq",))
                        prev = (kind, q, xT)
                    pk, pq, pxT = prev
                    (emit_proj_kv if pk == "kv" else emit_proj_q)(pq, pxT)

            # ---- Phase D: attention, per q-tile ----
            with tc.tile_pool(name="attn", bufs=3) as attn:
                state = {}

                def emit_scores(i):
                    L = (i + 1) * P
                    # chunk widths: keep every chunk >= 256 (fp32r runs
                    # 4 cyc/row below 256) except the unavoidable L=128 case
                    widths = []
                    rem = L
                    while rem > 640:
                        widths.append(512)
                        rem -= 512
                    if rem == 640:
                        widths += [384, 256]
                    else:
                        widths.append(rem)  # 128, 256, 384 or 512
                    s_t = attn.tile([P, S], F32, tag="s_sb")
                    off = 0
                    for c, w in enumerate(widths):
                        ps = ps_tile("mm", 512, F32)
                        last_chunk = c == len(widths) - 1
                        for kt in range(KT):
                            nc.tensor.matmul(
                                ps[:, :w],
                                qT[:, kt, i * P:(i + 1) * P],
                                kT[:, kt, off:off + w],
                                start=(kt == 0),
                                stop=(kt == KT - 1 and not last_chunk),
                            )
                        if last_chunk:
                            # add causal mask to the diagonal 128 cols via PE
                            nc.tensor.matmul(
                                ps[:, w - P:w], ident_bf, mask_bf,
                                start=False, stop=True,
                            )
                        nc.scalar.copy(s_t[:, off:off + w], ps[:, :w])
                        off += w
                    nmx = attn.tile([P, 1], F32, tag="nmx")
                    nc.vector.tensor_reduce(
                        out=nmx, in_=s_t[:, :L], axis=mybir.AxisListType.X,
                        op=mybir.AluOpType.max, negate=True,
                    )
                    p_t = attn.tile([P, S], F32R, tag="p_sb")
                    rs = attn.tile([P, 1], F32, tag="rs")
                    nc.scalar.activation(
                        out=p_t[:, :L], in_=s_t[:, :L],
                        func=mybir.ActivationFunctionType.Exp,
                        bias=nmx, scale=1.0, accum_out=rs,
                    )
                    rinv = attn.tile([P, 1], F32, tag="rinv")
                    nc.vector.reciprocal(rinv, rs)
                    state[i] = (p_t, rinv)

                def emit_out(i):
                    p_t, rinv = state.pop(i)
                    pT = attn.tile([P, ST, P], F32R, tag="pT")
                    for a in range((i + 4) // 4):
                        hi = min(4, i + 1 - 4 * a)
                        ps = ps_tile4("tp", F32R)
                        for j in range(hi):
                            st = 4 * a + j
                            nc.tensor.transpose(
                                ps[:, j, :], p_t[:, st * P:(st + 1) * P], ident_r
                            )
                        nc.vector.tensor_copy(
                            pT[:, 4 * a:4 * a + hi, :], ps[:, :hi, :]
                        )
                    ps_o = ps_tile("o", 512, F32)
                    for st in range(i + 1):
                        nc.tensor.matmul(
                            ps_o, pT[:, st, :], v_sb[:, st, :],
                            start=(st == 0), stop=(st == i),
                        )
                    o_t = attn.tile([P, DV], F32, tag="o_sb")
                    nc.scalar.activation(
                        out=o_t, in_=ps_o,
                        func=mybir.ActivationFunctionType.Copy, scale=rinv,
                    )
                    nc.sync.dma_start(out=out_d[i * P:(i + 1) * P, :], in_=o_t)

                LOOKAHEAD = 2
                for i in range(ST):
                    emit_scores(i)
                    if i >= LOOKAHEAD:
                        emit_out(i - LOOKAHEAD)
                for i in range(ST - LOOKAHEAD, ST):
                    emit_out(i)

    nc.finalize()
    return nc


_NC = None


def _get_nc():
    global _NC
    if _NC is None:
        _NC = _build()
    return _NC


def kernel(source_query, source_key_value, source_query_padding_mask,
           source_key_value_padding_mask, Wq, Wk, Wv):
    nc = _get_nc()
    wq = np.ascontiguousarray(Wq, dtype=np.float32)
    wk = np.ascontiguousarray(Wk, dtype=np.float32)
    wv = np.ascontiguousarray(Wv, dtype=np.float32)
    in_maps = [
        {
            "xq": np.ascontiguousarray(source_query[c], dtype=np.float32),
            "xkv": np.ascontiguousarray(source_key_value[c], dtype=np.float32),
            "wq": wq, "wk": wk, "wv": wv,
        }
        for c in range(N_CORES)
    ]
    try:
        res = run_bass_kernel_spmd(nc, in_maps, list(range(N_CORES)))
    except Exception:
        # transient NRT device errors have been observed through the axon
        # tunnel; one retry is usually enough
        res = run_bass_kernel_spmd(nc, in_maps, list(range(N_CORES)))
    return np.stack([res.results[c]["out"] for c in range(N_CORES)]).astype(np.float32)
